# revision 21
# baseline (speedup 1.0000x reference)
"""Trainium2 Bass kernel for nn_DeformConv2d_3246995276085.

Structural insight (see git history): the reference feeds pixel-space
coordinates into a grid_sample expecting normalized [-1,1] coords with
swapped axes, so only corner pixels (i, j <= 10) of each image ever
produce nonzero samples, and only scrambled-slab q=0 is live.  Output is
nonzero only at rows {9i-1..9i+2}; everything else is exactly zero.

Sharding: 8 cores = 4 images x 2 strip-halves (i in [0,6) / [6,12)).

This version is latency-optimized around the cost structure of TRN2
DMA (each hop ~2.7us: SEQ+HWDGE+DGE+sem-prop):
 - d-major gather stream j = 128*d + pix: bilinear corner weights stay
   in [pix, d] layout and apply as per-partition scalars (no weight
   DRAM round trip at all).
 - The 16-wrapped gather-index layout is produced ON CHIP by a PE
   partition-fold (8 selector matmuls + replicate matmul), no DRAM
   round trip for indices either.
 - One gather: host image xh2 packs channel pairs of vertically
   adjacent padded rows, so a single 512B element carries all four
   bilinear corners (y0/y1 x x0/x1); an INBY mask replaces the
   separate y1 clip.
 - Modulation is computed in feat order, replicated across channel
   partitions by a PE ones-matmul, and multiplied into the compact
   feat tile post-scatter.
"""

import functools

import numpy as np

ND = 9
C = 64
H = W = 96
NJ = 11          # j extent of corner region
NSTRIP = 6       # strip-rows (i values) per core
NPIX = 128       # padded corner-pixel domain (66 real + 62 dummy)
NIDX = NPIX * ND  # 1152 gather elements
SL = NIDX // 16   # 72 idx columns (wrapped-16)
XH2ROWS = 9606    # padded row-pair HWC image rows (98*98 + 2 spare)
DUMMY_BASE = 1.0e5

DIRY = np.array([0, 0, 0, 1, 1, 1, -1, -1, -1], np.float32)
DIRX = np.array([0, 1, -1, 0, 1, -1, 0, 1, -1], np.float32)

# fp32 conv blob [128, CWCOLS]
CW_XW2 = 0             # [128, 8*13] row-pair corner window
CW_WOFF2 = 104         # [128, 3*36] dy-pair offset weights (dx major)
CW_WOFF1 = 212         # [64, 3*36] dy=+1 singles
CW_SUMM = 320          # [36, 18] o1+o2 summing matrix
CW_BOFF = 338          # [36, 1] scaled conv biases
CW_BGX = 339           # [128, 9]  48*(ii+DIRY)+47.5
CW_BGY = 348           # [128, 9]  48*(jj+DIRX)+47.5
CWCOLS = 357

# fp32 misc blob [128, MICOLS]
MI_REPL = 0            # [16, 128]
MI_IDF = 128           # [128, 128] f32 identity
MI_BMOD = 256          # [1, 1]
MICOLS = 257

# bf16 blob [128, F16COLS]
B_XM2 = 0              # [128, 6*4*98] mod row-pair windows
B_IDB = 2352           # [128, 128] bf16 identity
B_WCNV = 2480          # [64, 9*64]
B_WMOD2 = 3056         # [128, 3] mod ty-pair weights
B_WMOD1 = 3059         # [64, 3] mod ty=2 singles
B_ONES = 3062          # [1, 64]
F16COLS = 3126


# ----------------------------------------------------------------- host prep

def _make_xh2(xb, bf16):
    """xb (64, 96, 96) -> row-pair HWC (XH2ROWS, 128) bf16: padded canvas
    rows yp and yp+1 channel-concatenated; pixel (yp, xp) at row yp*98+xp."""
    canvas = np.zeros((99, 98, C), np.float32)
    canvas[1:97, 1:97, :] = xb.transpose(1, 2, 0)
    out = np.zeros((XH2ROWS, 2 * C), bf16)
    v = out[:9604].reshape(98, 98, 2 * C)
    v[:, :, 0:C] = canvas[0:98].astype(bf16)
    v[:, :, C:2 * C] = canvas[1:99].astype(bf16)
    return out


def _make_core_inputs(x, w_off1, b_off1, w_off2, b_off2, w_mod, b_mod,
                      conv_weight, alpha, b, part):
    import ml_dtypes
    bf16 = ml_dtypes.bfloat16
    i0 = 6 * part
    xb = x[b]
    a1 = np.float32(48.0 * alpha)
    a2 = np.float32(48.0 * (1.0 - alpha))

    convw = np.zeros((128, CWCOLS), np.float32)
    # xw2: row-pair corner windows; rows r=0..7 hold x rows i0-1+r (lower)
    # and i0+r (upper half)
    xw2 = np.zeros((128, 8, 13), np.float32)
    for r in range(8):
        xr = i0 - 1 + r
        if 0 <= xr < H:
            xw2[0:64, r, 1:12] = xb[:, xr, 0:NJ]
        if 0 <= xr + 1 < H:
            xw2[64:128, r, 1:12] = xb[:, xr + 1, 0:NJ]
    convw[:, CW_XW2:CW_XW2 + 104] = xw2.reshape(128, 104)
    # woff scaled: channels 0:18 by 48*alpha (off1), 18:36 by 48*(1-alpha)
    wsc = np.concatenate([w_off1 * a1, w_off2 * a2], 0)  # (36, C, 3, 3)
    woff2 = np.zeros((128, 3, 36), np.float32)
    woff1 = np.zeros((64, 3, 36), np.float32)
    for dx in range(3):
        woff2[0:64, dx, :] = wsc[:, :, 0, dx].T   # dy=0 tap (lower=row ii-1)
        woff2[64:128, dx, :] = wsc[:, :, 1, dx].T  # dy=1 tap (upper=row ii)
        woff1[:, dx, :] = wsc[:, :, 2, dx].T       # dy=2 tap
    convw[:, CW_WOFF2:CW_WOFF2 + 108] = woff2.reshape(128, 108)
    convw[0:64, CW_WOFF1:CW_WOFF1 + 108] = woff1.reshape(64, 108)
    summ = np.zeros((36, 18), np.float32)
    for d in range(ND):
        summ[d, d] = 1.0
        summ[18 + d, d] = 1.0
        summ[9 + d, 9 + d] = 1.0
        summ[27 + d, 9 + d] = 1.0
    convw[0:36, CW_SUMM:CW_SUMM + 18] = summ
    convw[0:36, CW_BOFF] = np.concatenate(
        [b_off1 * a1, b_off2 * a2]).astype(np.float32)

    bgx = np.full((NPIX, ND), DUMMY_BASE, np.float32)
    bgy = np.full((NPIX, ND), DUMMY_BASE, np.float32)
    for p in range(NSTRIP * NJ):
        ii, jj = i0 + p // NJ, p % NJ
        bgx[p] = ii + DIRY
        bgy[p] = jj + DIRX
    convw[:, CW_BGX:CW_BGX + ND] = bgx * 48.0 + 47.5
    convw[:, CW_BGY:CW_BGY + ND] = bgy * 48.0 + 47.5

    misc = np.zeros((128, MICOLS), np.float32)
    misc[0:16, MI_REPL:MI_REPL + 128] = (
        np.arange(128)[None, :] % 16 == np.arange(16)[:, None])
    misc[:, MI_IDF:MI_IDF + 128] = np.eye(128, dtype=np.float32)
    misc[0, MI_BMOD] = np.float32(b_mod[0])

    # xm2: mod conv row-pair windows: lower r = x row 9s-1+r, upper = 9s+r
    xm2 = np.zeros((128, NSTRIP, 4, 98), np.float32)
    for s in range(NSTRIP):
        for r in range(4):
            xr = 9 * (i0 + s) - 1 + r
            if 0 <= xr < H:
                xm2[0:64, s, r, 1:97] = xb[:, xr, :]
            if 0 <= xr + 1 < H:
                xm2[64:128, s, r, 1:97] = xb[:, xr + 1, :]
    wcnv = np.zeros((C, ND, 64), np.float32)
    for t in range(9):
        dy, dx = t // 3, t % 3
        wcnv[:, t, :] = conv_weight[:, :, dy, dx].T
    blob16 = np.zeros((128, F16COLS), bf16)
    blob16[:, B_XM2:B_XM2 + 2352] = xm2.reshape(128, 2352).astype(bf16)
    blob16[:, B_IDB:B_IDB + 128] = np.eye(128, dtype=np.float32).astype(bf16)
    blob16[0:64, B_WCNV:B_WCNV + 576] = wcnv.reshape(C, 576).astype(bf16)
    wm2 = np.zeros((128, 3), np.float32)
    wm1 = np.zeros((64, 3), np.float32)
    for dx in range(3):
        wm2[0:64, dx] = w_mod[0, :, 0, dx]
        wm2[64:128, dx] = w_mod[0, :, 1, dx]
        wm1[:, dx] = w_mod[0, :, 2, dx]
    blob16[:, B_WMOD2:B_WMOD2 + 3] = wm2.astype(bf16)
    blob16[0:64, B_WMOD1:B_WMOD1 + 3] = wm1.astype(bf16)
    blob16[0:1, B_ONES:B_ONES + 64] = np.ones((1, 64), bf16)

    return {
        "xh2": _make_xh2(xb, bf16),
        "convw": convw,
        "misc": misc,
        "blob16": blob16,
    }


# ------------------------------------------------------------- device kernel

def emit_kernel(tc, outs, ins):
    from contextlib import ExitStack

    import concourse.bass as bass
    from concourse import mybir

    ctx = ExitStack()

    dt = mybir.dt
    Alu = mybir.AluOpType
    Act = mybir.ActivationFunctionType
    nc = tc.nc
    f32 = dt.float32
    bf = dt.bfloat16

    xh2 = ins["xh2"]
    strips_out = outs["strips_out"]

    consts = ctx.enter_context(tc.tile_pool(name="consts", bufs=1))
    work = ctx.enter_context(tc.tile_pool(name="work", bufs=1))
    loop_sb = ctx.enter_context(tc.tile_pool(name="loop_sb", bufs=3))
    psA = ctx.enter_context(tc.tile_pool(name="psA", bufs=1, space="PSUM"))
    psB = ctx.enter_context(tc.tile_pool(name="psB", bufs=2, space="PSUM"))
    psT = ctx.enter_context(tc.tile_pool(name="psT", bufs=1, space="PSUM"))
    psD = ctx.enter_context(tc.tile_pool(name="psD", bufs=3, space="PSUM"))

    def ap(t, offset_extra, dims):
        base = t[:] if not isinstance(t, bass.AP) else t
        return bass.AP(tensor=base.tensor, offset=base.offset + offset_extra,
                       ap=dims)

    # ---- input loads (SP carries fp32, Act carries bf16)
    CONVW = consts.tile([128, CWCOLS], f32)
    nc.sync.dma_start(out=CONVW, in_=ins["convw"])
    MISC = consts.tile([128, MICOLS], f32)
    nc.sync.dma_start(out=MISC, in_=ins["misc"])
    BLOB16 = consts.tile([128, F16COLS], bf)
    nc.scalar.dma_start(out=BLOB16, in_=ins["blob16"])

    XW2 = CONVW[:, CW_XW2:CW_XW2 + 104].rearrange("p (a b) -> p a b", a=8)
    WOFF2 = CONVW[:, CW_WOFF2:CW_WOFF2 + 108].rearrange(
        "p (a b) -> p a b", a=3)
    WOFF1 = CONVW[0:64, CW_WOFF1:CW_WOFF1 + 108].rearrange(
        "p (a b) -> p a b", a=3)
    SUMM = CONVW[0:36, CW_SUMM:CW_SUMM + 18]
    BOFF = CONVW[0:36, CW_BOFF:CW_BOFF + 1]
    BGX = CONVW[:, CW_BGX:CW_BGX + ND]
    BGY = CONVW[:, CW_BGY:CW_BGY + ND]
    REPL = MISC[0:16, MI_REPL:MI_REPL + 128]
    IDF = MISC[:, MI_IDF:MI_IDF + 128]
    BMOD = MISC[0:1, MI_BMOD:MI_BMOD + 1]
    XM2 = BLOB16[:, B_XM2:B_XM2 + 2352].rearrange(
        "p (s r c) -> p s r c", s=6, r=4)
    IDB = BLOB16[:, B_IDB:B_IDB + 128]
    WCNV = BLOB16[0:64, B_WCNV:B_WCNV + 576].rearrange("p (a b) -> p a b", a=9)
    WMOD2 = BLOB16[:, B_WMOD2:B_WMOD2 + 3]
    WMOD1 = BLOB16[0:64, B_WMOD1:B_WMOD1 + 3]
    ONES = BLOB16[0:1, B_ONES:B_ONES + 64]

    # ---- compact feat tile (only live rows {9s, 9s+1}) and zero conv rhs
    FP = work.tile([C, NSTRIP, 2, 98], bf)
    nc.gpsimd.memset(FP, 0.0)

    with tc.high_priority():
        # ---- corner offset conv (dy-paired) -> psum [36, 66] fp32
        ps_off = psA.tile([36, 66], f32, tag="ps_off")
        for dx in range(3):
            nc.tensor.matmul(ps_off, lhsT=WOFF2[:, dx, :],
                             rhs=XW2[:, 0:6, dx:dx + 11],
                             start=(dx == 0), stop=False)
        for dx in range(3):
            nc.tensor.matmul(ps_off, lhsT=WOFF1[:, dx, :],
                             rhs=XW2[0:64, 2:8, dx:dx + 11],
                             start=False, stop=(dx == 2))
        OFFS = work.tile([36, 66], f32)
        nc.vector.tensor_scalar(OFFS, ps_off, BOFF, None, Alu.add)

        # transpose + o1/o2 sum in one matmul: OCTS[pix, 0:9]=x, [9:18]=y
        ps_oc = psA.tile([66, 18], f32, tag="ps_off")
        nc.tensor.matmul(ps_oc, lhsT=OFFS, rhs=SUMM, start=True, stop=True)
        OCT = work.tile([NPIX, 18], f32)
        nc.vector.memset(OCT, 0.0)
        nc.vector.tensor_copy(OCT[0:66, :], ps_oc)

        # ---- pixel coords, x|y fused [128, 18] (host pre-scaled by 48
        # with +47.5 folded into BGX/BGY)
        IXY = work.tile([NPIX, 18], f32)
        nc.vector.tensor_add(IXY, OCT, CONVW[:, CW_BGX:CW_BGX + 18])
        TI = work.tile([NPIX, 18], dt.int32)
        nc.vector.tensor_copy(TI, IXY)
        TF = work.tile([NPIX, 18], f32)
        nc.vector.tensor_copy(TF, TI)
        GT = work.tile([NPIX, 18], f32)
        nc.vector.tensor_tensor(GT, TF, IXY, Alu.is_gt)
        XY0 = work.tile([NPIX, 18], f32)
        nc.vector.tensor_sub(XY0, TF, GT)
        FXY = work.tile([NPIX, 18], f32)
        nc.vector.tensor_sub(FXY, IXY, XY0)
        CXY = work.tile([NPIX, 18], f32)
        nc.vector.tensor_scalar(CXY, XY0, -1.0, 96.0, Alu.max, Alu.min)
        IX0 = XY0[:, 0:9]
        IY0 = XY0[:, 9:18]
        FX = FXY[:, 0:9]
        FY = FXY[:, 9:18]
        QI = work.tile([NPIX, ND], f32)
        nc.vector.scalar_tensor_tensor(QI, CXY[:, 9:18], 98.0, CXY[:, 0:9],
                                       Alu.mult, Alu.add)
        nc.vector.tensor_scalar(QI, QI, 99.0, None, Alu.add)

        # ---- on-chip idx fold to the 16-wrapped d-major gather layout:
        # idx slot j = 128*d + 16*a + r  ->  IDXC[16k+r, 8*d+a]
        PSI = psA.tile([16, 8, ND], f32, tag="ps_off")
        for a in range(8):
            nc.tensor.matmul(PSI[:, a, :], lhsT=IDF[:, 16 * a:16 * a + 16],
                             rhs=QI, start=True, stop=True)
        IDXF = work.tile([16, ND, 8], f32)
        nc.vector.tensor_copy(IDXF, PSI[:].rearrange("p a d -> p d a"))
        ps2 = psA.tile([128, SL], f32, tag="ps_off")
        nc.tensor.matmul(ps2, lhsT=REPL, rhs=IDXF, start=True, stop=True)
        IDXC = work.tile([128, SL], dt.int16)
        nc.vector.tensor_copy(IDXC, ps2)

        # ---- single gather: element = row-pair pixel (4 corners, 256 bf16)
        xh2_src = bass.AP(tensor=xh2.tensor, offset=xh2.offset,
                          ap=[[128, 9604], [1, 256]])
        VV1 = work.tile([128, 5, 256], bf)
        nc.gpsimd.dma_gather(out_ap=VV1, in_ap=xh2_src,
                             idxs_ap=IDXC[:, 0:40],
                             num_idxs=5 * 128, num_idxs_reg=5 * 128,
                             elem_size=256, elem_step=128,
                             single_packet=False)
        VV2 = work.tile([128, 4, 256], bf)
        nc.gpsimd.dma_gather(out_ap=VV2, in_ap=xh2_src,
                             idxs_ap=IDXC[:, 40:72],
                             num_idxs=4 * 128, num_idxs_reg=4 * 128,
                             elem_size=256, elem_step=128,
                             single_packet=False)

    # ---- modulation conv in feat order (ty-paired) -> sigmoid -> MODV
    MODV = work.tile([1, NSTRIP, 99], bf)
    for c2 in range(2):
        ps_m = psB.tile([1, 3, 96], f32, tag="ps_m")
        for dx in range(3):
            nc.tensor.matmul(ps_m, lhsT=WMOD2[:, dx:dx + 1],
                             rhs=XM2[:, 3 * c2:3 * c2 + 3, 0:1, dx:96 + dx],
                             start=(dx == 0), stop=False)
        for dx in range(3):
            nc.tensor.matmul(ps_m, lhsT=WMOD1[:, dx:dx + 1],
                             rhs=XM2[0:64, 3 * c2:3 * c2 + 3, 2:3, dx:96 + dx],
                             start=False, stop=(dx == 2))
        nc.scalar.activation(MODV[:, 3 * c2:3 * c2 + 3, 0:96], ps_m,
                             Act.Sigmoid, bias=BMOD, scale=1.0)
    ps_m2 = psB.tile([1, NSTRIP, 3], f32, tag="ps_m")
    for dx in range(3):
        nc.tensor.matmul(ps_m2, lhsT=WMOD2[:, dx:dx + 1],
                         rhs=XM2[:, :, 1:2, dx:3 + dx],
                         start=(dx == 0), stop=False)
    for dx in range(3):
        nc.tensor.matmul(ps_m2, lhsT=WMOD1[:, dx:dx + 1],
                         rhs=XM2[0:64, :, 3:4, dx:3 + dx],
                         start=False, stop=(dx == 2))
    nc.scalar.activation(MODV[:, :, 96:99], ps_m2, Act.Sigmoid,
                         bias=BMOD, scale=1.0)

    # replicate mod across the 64 channel partitions (PE ones-matmul)
    MODR = work.tile([C, NSTRIP, 99], bf)
    psM1 = psB.tile([C, 297], f32, tag="ps_m")
    nc.tensor.matmul(psM1, lhsT=ONES,
                     rhs=ap(MODV, 0, [MODV[:].ap[0], [1, 297]]),
                     start=True, stop=True)
    psM2 = psB.tile([C, 297], f32, tag="ps_m")
    nc.tensor.matmul(psM2, lhsT=ONES,
                     rhs=ap(MODV, 297, [MODV[:].ap[0], [1, 297]]),
                     start=True, stop=True)
    nc.scalar.copy(ap(MODR, 0, [MODR[:].ap[0], [1, 297]]), psM1)
    nc.scalar.copy(ap(MODR, 297, [MODR[:].ap[0], [1, 297]]), psM2)

    # ---- bilinear corner weights as per-(pix,d) scalars (DVE, in the
    # gather window); bf16 outputs for the bf16 combine
    CB = work.tile([NPIX, 18], f32)
    nc.vector.tensor_scalar(CB, XY0, -1.0, None, Alu.is_ge)
    INB = work.tile([NPIX, 18], f32)
    nc.vector.scalar_tensor_tensor(INB, XY0, 96.0, CB, Alu.is_le, Alu.mult)
    W0 = work.tile([NPIX, 18], f32)
    nc.vector.tensor_scalar(W0, FXY, -1.0, 1.0, Alu.mult, Alu.add)
    A0 = work.tile([NPIX, ND], f32)
    nc.vector.tensor_mul(A0, W0[:, 0:9], INB[:, 0:9])
    A1 = work.tile([NPIX, ND], f32)
    nc.vector.tensor_mul(A1, FX, INB[:, 0:9])
    Y0 = work.tile([NPIX, ND], f32)
    nc.vector.tensor_mul(Y0, W0[:, 9:18], INB[:, 9:18])
    Y1 = work.tile([NPIX, ND], f32)
    nc.vector.tensor_mul(Y1, FY, INB[:, 9:18])
    CW = work.tile([NPIX, 4, ND], bf)
    nc.vector.tensor_mul(CW[:, 0, :], Y0, A0)   # (y0, x0)
    nc.vector.tensor_mul(CW[:, 1, :], Y1, A0)   # (y1, x0)
    nc.vector.tensor_mul(CW[:, 2, :], Y0, A1)   # (y0, x1)
    nc.vector.tensor_mul(CW[:, 3, :], Y1, A1)   # (y1, x1)

    # expand corner weights along ch on Act (idle in the gather window) so
    # the combine hits DVE 2-byte fast mode (all last dims packed)
    CWE = work.tile([NPIX, 4, ND, 64], bf)
    for c in range(4):
        nc.scalar.copy(CWE[:, c, :, :],
                       ap(CW, ND * c, [CW[:].ap[0], [1, ND], [0, 64]]))

    # ---- weighted corner combine per gather half (separate tiles so the
    # first half combines while the second gather is in flight)
    TC1 = work.tile([NPIX, 5, 64], bf)
    TC2 = work.tile([NPIX, 4, 64], bf)

    def halves(c, lo, hi, vv):
        vs = ap(vv, 64 * c, [vv[:].ap[0], [256, hi - lo], [1, 64]])
        ws = CWE[:, c, lo:hi, :]
        return vs, ws

    for (lo, hi, vv, tcx, gtag) in ((0, 5, VV1, TC1, "g1"),
                                    (5, 9, VV2, TC2, "g2")):
        n = hi - lo
        m1 = work.tile([NPIX, n, 64], bf, tag=f"m1{gtag}")
        m2 = work.tile([NPIX, n, 64], bf, tag=f"m2{gtag}")
        m3 = work.tile([NPIX, n, 64], bf, tag=f"m3{gtag}")
        v0, w0 = halves(0, lo, hi, vv)
        v1, w1 = halves(1, lo, hi, vv)
        v2, w2 = halves(2, lo, hi, vv)
        v3, w3 = halves(3, lo, hi, vv)
        nc.vector.tensor_tensor(tcx, v0, w0, Alu.mult)
        nc.vector.tensor_tensor(m1, v1, w1, Alu.mult)
        nc.vector.tensor_tensor(m2, v2, w2, Alu.mult)
        nc.vector.tensor_tensor(m3, v3, w3, Alu.mult)
        nc.vector.tensor_add(tcx, tcx, m1)
        nc.vector.tensor_add(m2, m2, m3)
        nc.vector.tensor_add(tcx, tcx, m2)

    # ---- per-d transpose to [ch, pix]
    psTA = psT.tile([C, 8, 128], bf, tag="ta")
    psTB = psT.tile([C, 1, 128], bf, tag="tb")
    for d in range(9):
        pd = psTA[:, d, :] if d < 8 else psTB[:, 0, :]
        tcs = TC1[:, d, :] if d < 5 else TC2[:, d - 5, :]
        nc.tensor.transpose(pd, tcs, IDB)

    # ---- fused scatter+modulation into FP: feat col t = 9*jj + d
    fpap = FP[:].ap[0]
    mdap = MODR[:].ap[0]
    taap = psTA[:].ap[0]
    tbap = psTB[:].ap[0]
    # d 0..4, jj 0..10 (phi=0 cols 1+9jj+d) -- DVE
    nc.vector.tensor_tensor(
        ap(FP, 1, [fpap, [196, 6], [1, 5], [9, NJ]]),
        ap(psTA, 0, [taap, [NJ, 6], [128, 5], [1, NJ]]),
        ap(MODR, 0, [mdap, [99, 6], [1, 5], [9, NJ]]), Alu.mult)
    # d 5, jj 0..10 -- DVE
    nc.vector.tensor_tensor(
        ap(FP, 6, [fpap, [196, 6], [9, NJ]]),
        ap(psTA, 5 * 128, [taap, [NJ, 6], [1, NJ]]),
        ap(MODR, 5, [mdap, [99, 6], [9, NJ]]), Alu.mult)
    # d 6..7, jj 0..9
    nc.vector.tensor_tensor(
        ap(FP, 7, [fpap, [196, 6], [1, 2], [9, 10]]),
        ap(psTA, 6 * 128, [taap, [NJ, 6], [128, 2], [1, 10]]),
        ap(MODR, 6, [mdap, [99, 6], [1, 2], [9, 10]]), Alu.mult)
    # d 8, jj 0..9
    nc.vector.tensor_tensor(
        ap(FP, 9, [fpap, [196, 6], [9, 10]]),
        ap(psTB, 0, [tbap, [NJ, 6], [1, 10]]),
        ap(MODR, 8, [mdap, [99, 6], [9, 10]]), Alu.mult)
    # phi=1 fixups: t in {96, 97, 98} from (d, jj) = (6..8, 10)
    nc.vector.tensor_tensor(
        ap(FP, 98 + 1, [fpap, [196, 6], [1, 2]]),
        ap(psTA, 6 * 128 + 10, [taap, [NJ, 6], [128, 2]]),
        ap(MODR, 96, [mdap, [99, 6], [1, 2]]), Alu.mult)
    nc.vector.tensor_tensor(
        ap(FP, 98 + 3, [fpap, [196, 6], [1, 1]]),
        ap(psTB, 10, [tbap, [NJ, 6], [1, 1]]),
        ap(MODR, 98, [mdap, [99, 6], [1, 1]]), Alu.mult)

    # ---- final conv strips: tap-accumulate over the 2 live feat rows;
    # feat row 9s+phi feeds out row 9s+phi-dy, i.e. dst rows (1-dy):(3-dy).
    # tap order (dy=+1, dy=-1, dy=0): the first tap of each dy group
    # start=True-initializes its disjoint 2-row region (rows 0:2 then 2:4),
    # so no zero-priming matmul is needed.
    TAP_ORDER = (6, 7, 8, 0, 1, 2, 3, 4, 5)
    for s in range(NSTRIP):
        ps_c = psD.tile([C, 4, 96], f32, tag="ps_c")
        for i, t in enumerate(TAP_ORDER):
            dy, dx = t // 3 - 1, t % 3 - 1
            nc.tensor.matmul(
                ps_c[:, 1 - dy:3 - dy, :],
                lhsT=WCNV[:, t, :],
                rhs=FP[:, s, :, 1 + dx:97 + dx],
                start=(i in (0, 3)),
                stop=(i in (2, 5, 8)),
                skip_group_check=True,
            )
        if s % 2 == 0:
            OUTS2 = loop_sb.tile([C, 2, 4, 96], bf, tag="outs")
        nc.scalar.copy(OUTS2[:, s % 2, 0:2, :], ps_c[:, 0:2, :])
        nc.vector.tensor_copy(OUTS2[:, s % 2, 2:4, :], ps_c[:, 2:4, :])
        if s % 2 == 1:
            if s % 4 == 1:
                nc.sync.dma_start(out=strips_out[:, s - 1:s + 1], in_=OUTS2)
            else:
                nc.scalar.dma_start(out=strips_out[:, s - 1:s + 1], in_=OUTS2)

    # PE p-state warmers: tiny no-op matmuls the scheduler slots into PE
    # idle gaps so the tensor engine stays at full clock for the
    # transposes and final conv strips
    ps_w = psA.tile([C, 64], f32, tag="ps_off")
    with tc.high_priority(offset=-1000000):
        for _ in range(200):
            nc.tensor.matmul(ps_w, lhsT=IDB[0:64, 0:64],
                             rhs=IDB[0:64, 64:128], start=True, stop=True)

    ctx.close()


@functools.lru_cache(maxsize=1)
def _build_program():
    from contextlib import ExitStack

    import concourse.bacc as bacc
    import concourse.tile as tile
    from concourse import mybir

    dt = mybir.dt
    nc = bacc.Bacc("TRN2", target_bir_lowering=False, debug=False)
    ins = {
        "xh2": nc.dram_tensor("xh2", [XH2ROWS, 2 * C], dt.bfloat16,
                              kind="ExternalInput").ap(),
        "convw": nc.dram_tensor("convw", [128, CWCOLS], dt.float32,
                                kind="ExternalInput").ap(),
        "misc": nc.dram_tensor("misc", [128, MICOLS], dt.float32,
                               kind="ExternalInput").ap(),
        "blob16": nc.dram_tensor("blob16", [128, F16COLS], dt.bfloat16,
                                 kind="ExternalInput").ap(),
    }
    outs = {
        "strips_out": nc.dram_tensor("strips_out", [C, NSTRIP, 4, 96],
                                     dt.bfloat16, kind="ExternalOutput").ap(),
    }
    with ExitStack() as ctx:
        tc = ctx.enter_context(tile.TileContext(nc))
        emit_kernel(tc, outs, ins)
    nc.compile()
    return nc


def _host_inputs(inputs):
    arrs = {k: np.asarray(v, np.float32) for k, v in inputs.items()}
    in_maps = []
    for core in range(8):
        b, part = core // 2, core % 2
        in_maps.append(_make_core_inputs(
            arrs["x"], arrs["w_off1"], arrs["b_off1"], arrs["w_off2"],
            arrs["b_off2"], arrs["w_mod"], arrs["b_mod"],
            arrs["conv_weight"], float(arrs["alpha"][0]), b, part))
    return in_maps


def _assemble(results):
    out = np.zeros((4, C, H, W), np.float32)
    for core, res in enumerate(results):
        b, part = core // 2, core % 2
        i0 = 6 * part
        strips = np.asarray(res["strips_out"], dtype=np.float32)
        for s in range(NSTRIP):
            r0 = 9 * (i0 + s) - 1
            if r0 < 0:
                out[b][:, 0:r0 + 4, :] = strips[:, s, -r0:, :]
            elif r0 + 4 <= H:
                out[b][:, r0:r0 + 4, :] = strips[:, s]
    return out


def kernel(**inputs) -> np.ndarray:
    from concourse.bass_utils import run_bass_kernel_spmd

    nc = _build_program()
    in_maps = _host_inputs(inputs)
    res = run_bass_kernel_spmd(nc, in_maps, core_ids=list(range(8)))
    return _assemble(res.results)


if __name__ == "__main__":
    d = dict(np.load("/root/problem/inputs_cache.npz"))
    out = kernel(**d)
    ref = np.load("/root/problem/expected_np.npy")
    err = np.abs(out - ref).max()
    print("absmax err:", err, "rel:", err / np.abs(ref).max())


# revision 22
# speedup vs baseline: 1.0177x; 1.0177x over previous
"""Trainium2 Bass kernel for nn_DeformConv2d_3246995276085.

Structural insight (see git history): the reference feeds pixel-space
coordinates into a grid_sample expecting normalized [-1,1] coords with
swapped axes, so only corner pixels (i, j <= 10) of each image ever
produce nonzero samples, and only scrambled-slab q=0 is live.  Output is
nonzero only at rows {9i-1..9i+2}; everything else is exactly zero.

Sharding: 8 cores = 4 images x 2 strip-halves (i in [0,6) / [6,12)).

This version is latency-optimized around the cost structure of TRN2
DMA (each hop ~2.7us: SEQ+HWDGE+DGE+sem-prop):
 - d-major gather stream j = 128*d + pix: bilinear corner weights stay
   in [pix, d] layout and apply as per-partition scalars (no weight
   DRAM round trip at all).
 - The 16-wrapped gather-index layout is produced ON CHIP by a PE
   partition-fold (8 selector matmuls + replicate matmul), no DRAM
   round trip for indices either.
 - One gather: host image xh2 packs channel pairs of vertically
   adjacent padded rows, so a single 512B element carries all four
   bilinear corners (y0/y1 x x0/x1); an INBY mask replaces the
   separate y1 clip.
 - Modulation is computed in feat order, replicated across channel
   partitions by a PE ones-matmul, and multiplied into the compact
   feat tile post-scatter.
"""

import functools

import numpy as np

ND = 9
C = 64
H = W = 96
NJ = 11          # j extent of corner region
NSTRIP = 6       # strip-rows (i values) per core
NPIX = 128       # padded corner-pixel domain (66 real + 62 dummy)
NIDX = NPIX * ND  # 1152 gather elements
SL = NIDX // 16   # 72 idx columns (wrapped-16)
XH2ROWS = 9606    # padded row-pair HWC image rows (98*98 + 2 spare)
DUMMY_BASE = 1.0e5

DIRY = np.array([0, 0, 0, 1, 1, 1, -1, -1, -1], np.float32)
DIRX = np.array([0, 1, -1, 0, 1, -1, 0, 1, -1], np.float32)

# fp32 conv blob [128, CWCOLS]
CW_XW2 = 0             # [128, 8*13] row-pair corner window
CW_WOFF2 = 104         # [128, 3*36] dy-pair offset weights (dx major)
CW_WOFF1 = 212         # [64, 3*36] dy=+1 singles
CW_SUMM = 320          # [36, 18] o1+o2 summing matrix
CW_BOFF = 338          # [36, 1] scaled conv biases
CW_BGX = 339           # [128, 9]  48*(ii+DIRY)+47.5
CW_BGY = 348           # [128, 9]  48*(jj+DIRX)+47.5
CWCOLS = 357

# fp32 misc blob [128, MICOLS]
MI_REPL = 0            # [16, 128]
MI_IDF = 128           # [128, 128] f32 identity
MI_BMOD = 256          # [1, 1]
MICOLS = 257

# bf16 blob [128, F16COLS]
B_XM2 = 0              # [128, 6*4*98] mod row-pair windows
B_IDB = 2352           # [128, 128] bf16 identity
B_WCNV = 2480          # [64, 9*64]
B_WMOD2 = 3056         # [128, 3] mod ty-pair weights
B_WMOD1 = 3059         # [64, 3] mod ty=2 singles
B_ONES = 3062          # [1, 64]
F16COLS = 3126


# ----------------------------------------------------------------- host prep

def _make_xh2(xb, bf16):
    """xb (64, 96, 96) -> row-pair HWC (XH2ROWS, 128) bf16: padded canvas
    rows yp and yp+1 channel-concatenated; pixel (yp, xp) at row yp*98+xp."""
    canvas = np.zeros((99, 98, C), np.float32)
    canvas[1:97, 1:97, :] = xb.transpose(1, 2, 0)
    out = np.zeros((XH2ROWS, 2 * C), bf16)
    v = out[:9604].reshape(98, 98, 2 * C)
    v[:, :, 0:C] = canvas[0:98].astype(bf16)
    v[:, :, C:2 * C] = canvas[1:99].astype(bf16)
    return out


def _make_core_inputs(x, w_off1, b_off1, w_off2, b_off2, w_mod, b_mod,
                      conv_weight, alpha, b, part):
    import ml_dtypes
    bf16 = ml_dtypes.bfloat16
    i0 = 6 * part
    xb = x[b]
    a1 = np.float32(48.0 * alpha)
    a2 = np.float32(48.0 * (1.0 - alpha))

    convw = np.zeros((128, CWCOLS), np.float32)
    # xw2: row-pair corner windows; rows r=0..7 hold x rows i0-1+r (lower)
    # and i0+r (upper half)
    xw2 = np.zeros((128, 8, 13), np.float32)
    for r in range(8):
        xr = i0 - 1 + r
        if 0 <= xr < H:
            xw2[0:64, r, 1:12] = xb[:, xr, 0:NJ]
        if 0 <= xr + 1 < H:
            xw2[64:128, r, 1:12] = xb[:, xr + 1, 0:NJ]
    convw[:, CW_XW2:CW_XW2 + 104] = xw2.reshape(128, 104)
    # woff scaled: channels 0:18 by 48*alpha (off1), 18:36 by 48*(1-alpha)
    wsc = np.concatenate([w_off1 * a1, w_off2 * a2], 0)  # (36, C, 3, 3)
    woff2 = np.zeros((128, 3, 36), np.float32)
    woff1 = np.zeros((64, 3, 36), np.float32)
    for dx in range(3):
        woff2[0:64, dx, :] = wsc[:, :, 0, dx].T   # dy=0 tap (lower=row ii-1)
        woff2[64:128, dx, :] = wsc[:, :, 1, dx].T  # dy=1 tap (upper=row ii)
        woff1[:, dx, :] = wsc[:, :, 2, dx].T       # dy=2 tap
    convw[:, CW_WOFF2:CW_WOFF2 + 108] = woff2.reshape(128, 108)
    convw[0:64, CW_WOFF1:CW_WOFF1 + 108] = woff1.reshape(64, 108)
    summ = np.zeros((36, 18), np.float32)
    for d in range(ND):
        summ[d, d] = 1.0
        summ[18 + d, d] = 1.0
        summ[9 + d, 9 + d] = 1.0
        summ[27 + d, 9 + d] = 1.0
    convw[0:36, CW_SUMM:CW_SUMM + 18] = summ
    convw[0:36, CW_BOFF] = np.concatenate(
        [b_off1 * a1, b_off2 * a2]).astype(np.float32)

    bgx = np.full((NPIX, ND), DUMMY_BASE, np.float32)
    bgy = np.full((NPIX, ND), DUMMY_BASE, np.float32)
    for p in range(NSTRIP * NJ):
        ii, jj = i0 + p // NJ, p % NJ
        bgx[p] = ii + DIRY
        bgy[p] = jj + DIRX
    convw[:, CW_BGX:CW_BGX + ND] = bgx * 48.0 + 47.5
    convw[:, CW_BGY:CW_BGY + ND] = bgy * 48.0 + 47.5

    misc = np.zeros((128, MICOLS), np.float32)
    misc[0:16, MI_REPL:MI_REPL + 128] = (
        np.arange(128)[None, :] % 16 == np.arange(16)[:, None])
    misc[:, MI_IDF:MI_IDF + 128] = np.eye(128, dtype=np.float32)
    misc[0, MI_BMOD] = np.float32(b_mod[0])

    # xm2: mod conv row-pair windows: lower r = x row 9s-1+r, upper = 9s+r
    xm2 = np.zeros((128, NSTRIP, 4, 98), np.float32)
    for s in range(NSTRIP):
        for r in range(4):
            xr = 9 * (i0 + s) - 1 + r
            if 0 <= xr < H:
                xm2[0:64, s, r, 1:97] = xb[:, xr, :]
            if 0 <= xr + 1 < H:
                xm2[64:128, s, r, 1:97] = xb[:, xr + 1, :]
    wcnv = np.zeros((C, ND, 64), np.float32)
    for t in range(9):
        dy, dx = t // 3, t % 3
        wcnv[:, t, :] = conv_weight[:, :, dy, dx].T
    blob16 = np.zeros((128, F16COLS), bf16)
    blob16[:, B_XM2:B_XM2 + 2352] = xm2.reshape(128, 2352).astype(bf16)
    blob16[:, B_IDB:B_IDB + 128] = np.eye(128, dtype=np.float32).astype(bf16)
    blob16[0:64, B_WCNV:B_WCNV + 576] = wcnv.reshape(C, 576).astype(bf16)
    wm2 = np.zeros((128, 3), np.float32)
    wm1 = np.zeros((64, 3), np.float32)
    for dx in range(3):
        wm2[0:64, dx] = w_mod[0, :, 0, dx]
        wm2[64:128, dx] = w_mod[0, :, 1, dx]
        wm1[:, dx] = w_mod[0, :, 2, dx]
    blob16[:, B_WMOD2:B_WMOD2 + 3] = wm2.astype(bf16)
    blob16[0:64, B_WMOD1:B_WMOD1 + 3] = wm1.astype(bf16)
    blob16[0:1, B_ONES:B_ONES + 64] = np.ones((1, 64), bf16)

    return {
        "xh2": _make_xh2(xb, bf16),
        "convw": convw,
        "misc": misc,
        "blob16": blob16,
    }


# ------------------------------------------------------------- device kernel

def emit_kernel(tc, outs, ins):
    from contextlib import ExitStack

    import concourse.bass as bass
    from concourse import mybir

    ctx = ExitStack()

    dt = mybir.dt
    Alu = mybir.AluOpType
    Act = mybir.ActivationFunctionType
    nc = tc.nc
    f32 = dt.float32
    bf = dt.bfloat16

    xh2 = ins["xh2"]
    strips_out = outs["strips_out"]

    consts = ctx.enter_context(tc.tile_pool(name="consts", bufs=1))
    work = ctx.enter_context(tc.tile_pool(name="work", bufs=1))
    loop_sb = ctx.enter_context(tc.tile_pool(name="loop_sb", bufs=3))
    psA = ctx.enter_context(tc.tile_pool(name="psA", bufs=1, space="PSUM"))
    psB = ctx.enter_context(tc.tile_pool(name="psB", bufs=2, space="PSUM"))
    psT = ctx.enter_context(tc.tile_pool(name="psT", bufs=1, space="PSUM"))
    psD = ctx.enter_context(tc.tile_pool(name="psD", bufs=3, space="PSUM"))

    def ap(t, offset_extra, dims):
        base = t[:] if not isinstance(t, bass.AP) else t
        return bass.AP(tensor=base.tensor, offset=base.offset + offset_extra,
                       ap=dims)

    # ---- input loads (SP carries fp32, Act carries bf16)
    CONVW = consts.tile([128, CWCOLS], f32)
    nc.sync.dma_start(out=CONVW, in_=ins["convw"])
    MISC = consts.tile([128, MICOLS], f32)
    nc.sync.dma_start(out=MISC, in_=ins["misc"])
    BLOB16 = consts.tile([128, F16COLS], bf)
    nc.scalar.dma_start(out=BLOB16, in_=ins["blob16"])

    XW2 = CONVW[:, CW_XW2:CW_XW2 + 104].rearrange("p (a b) -> p a b", a=8)
    WOFF2 = CONVW[:, CW_WOFF2:CW_WOFF2 + 108].rearrange(
        "p (a b) -> p a b", a=3)
    WOFF1 = CONVW[0:64, CW_WOFF1:CW_WOFF1 + 108].rearrange(
        "p (a b) -> p a b", a=3)
    SUMM = CONVW[0:36, CW_SUMM:CW_SUMM + 18]
    BOFF = CONVW[0:36, CW_BOFF:CW_BOFF + 1]
    BGX = CONVW[:, CW_BGX:CW_BGX + ND]
    BGY = CONVW[:, CW_BGY:CW_BGY + ND]
    REPL = MISC[0:16, MI_REPL:MI_REPL + 128]
    IDF = MISC[:, MI_IDF:MI_IDF + 128]
    BMOD = MISC[0:1, MI_BMOD:MI_BMOD + 1]
    XM2 = BLOB16[:, B_XM2:B_XM2 + 2352].rearrange(
        "p (s r c) -> p s r c", s=6, r=4)
    IDB = BLOB16[:, B_IDB:B_IDB + 128]
    WCNV = BLOB16[0:64, B_WCNV:B_WCNV + 576].rearrange("p (a b) -> p a b", a=9)
    WMOD2 = BLOB16[:, B_WMOD2:B_WMOD2 + 3]
    WMOD1 = BLOB16[0:64, B_WMOD1:B_WMOD1 + 3]
    ONES = BLOB16[0:1, B_ONES:B_ONES + 64]

    # ---- compact feat tile (only live rows {9s, 9s+1}) and zero conv rhs
    FP = work.tile([C, NSTRIP, 2, 98], bf)
    nc.gpsimd.memset(FP, 0.0)

    with tc.high_priority():
        # ---- corner offset conv (dy-paired) -> psum [36, 66] fp32
        ps_off = psA.tile([36, 66], f32, tag="ps_off")
        for dx in range(3):
            nc.tensor.matmul(ps_off, lhsT=WOFF2[:, dx, :],
                             rhs=XW2[:, 0:6, dx:dx + 11],
                             start=(dx == 0), stop=False)
        for dx in range(3):
            nc.tensor.matmul(ps_off, lhsT=WOFF1[:, dx, :],
                             rhs=XW2[0:64, 2:8, dx:dx + 11],
                             start=False, stop=(dx == 2))
        OFFS = work.tile([36, 66], f32)
        nc.vector.tensor_scalar(OFFS, ps_off, BOFF, None, Alu.add)

        # transpose + o1/o2 sum in one matmul: OCTS[pix, 0:9]=x, [9:18]=y
        ps_oc = psA.tile([66, 18], f32, tag="ps_off")
        nc.tensor.matmul(ps_oc, lhsT=OFFS, rhs=SUMM, start=True, stop=True)
        OCT = work.tile([NPIX, 18], f32)
        nc.vector.memset(OCT, 0.0)
        nc.vector.tensor_copy(OCT[0:66, :], ps_oc)

        # ---- pixel coords, x|y fused [128, 18] (host pre-scaled by 48
        # with +47.5 folded into BGX/BGY)
        IXY = work.tile([NPIX, 18], f32)
        nc.vector.tensor_add(IXY, OCT, CONVW[:, CW_BGX:CW_BGX + 18])
        TI = work.tile([NPIX, 18], dt.int32)
        nc.vector.tensor_copy(TI, IXY)
        TF = work.tile([NPIX, 18], f32)
        nc.vector.tensor_copy(TF, TI)
        GT = work.tile([NPIX, 18], f32)
        nc.vector.tensor_tensor(GT, TF, IXY, Alu.is_gt)
        XY0 = work.tile([NPIX, 18], f32)
        nc.vector.tensor_sub(XY0, TF, GT)
        FXY = work.tile([NPIX, 18], f32)
        nc.vector.tensor_sub(FXY, IXY, XY0)
        CXY = work.tile([NPIX, 18], f32)
        nc.vector.tensor_scalar(CXY, XY0, -1.0, 96.0, Alu.max, Alu.min)
        IX0 = XY0[:, 0:9]
        IY0 = XY0[:, 9:18]
        FX = FXY[:, 0:9]
        FY = FXY[:, 9:18]
        QI = work.tile([NPIX, ND], f32)
        nc.vector.scalar_tensor_tensor(QI, CXY[:, 9:18], 98.0, CXY[:, 0:9],
                                       Alu.mult, Alu.add)
        nc.vector.tensor_scalar(QI, QI, 99.0, None, Alu.add)

        # ---- on-chip idx fold to the 16-wrapped d-major gather layout:
        # idx slot j = 128*d + 16*a + r  ->  IDXC[16k+r, 8*d+a]
        PSI = psA.tile([16, 8, ND], f32, tag="ps_off")
        for a in range(8):
            nc.tensor.matmul(PSI[:, a, :], lhsT=IDF[:, 16 * a:16 * a + 16],
                             rhs=QI, start=True, stop=True)
        IDXF = work.tile([16, ND, 8], f32)
        nc.vector.tensor_copy(IDXF, PSI[:].rearrange("p a d -> p d a"))
        ps2 = psA.tile([128, SL], f32, tag="ps_off")
        nc.tensor.matmul(ps2, lhsT=REPL, rhs=IDXF, start=True, stop=True)
        IDXC = work.tile([128, SL], dt.int16)
        nc.vector.tensor_copy(IDXC, ps2)

        # ---- single gather: element = row-pair pixel (4 corners, 256 bf16)
        xh2_src = bass.AP(tensor=xh2.tensor, offset=xh2.offset,
                          ap=[[128, 9604], [1, 256]])
        VV1 = work.tile([128, 6, 256], bf)
        nc.gpsimd.dma_gather(out_ap=VV1, in_ap=xh2_src,
                             idxs_ap=IDXC[:, 0:48],
                             num_idxs=6 * 128, num_idxs_reg=6 * 128,
                             elem_size=256, elem_step=128,
                             single_packet=False)
        VV2 = work.tile([128, 3, 256], bf)
        nc.gpsimd.dma_gather(out_ap=VV2, in_ap=xh2_src,
                             idxs_ap=IDXC[:, 48:72],
                             num_idxs=3 * 128, num_idxs_reg=3 * 128,
                             elem_size=256, elem_step=128,
                             single_packet=False)

    # ---- modulation conv in feat order (ty-paired) -> sigmoid -> MODV
    MODV = work.tile([1, NSTRIP, 99], bf)
    for c2 in range(2):
        ps_m = psB.tile([1, 3, 96], f32, tag="ps_m")
        for dx in range(3):
            nc.tensor.matmul(ps_m, lhsT=WMOD2[:, dx:dx + 1],
                             rhs=XM2[:, 3 * c2:3 * c2 + 3, 0:1, dx:96 + dx],
                             start=(dx == 0), stop=False)
        for dx in range(3):
            nc.tensor.matmul(ps_m, lhsT=WMOD1[:, dx:dx + 1],
                             rhs=XM2[0:64, 3 * c2:3 * c2 + 3, 2:3, dx:96 + dx],
                             start=False, stop=(dx == 2))
        nc.scalar.activation(MODV[:, 3 * c2:3 * c2 + 3, 0:96], ps_m,
                             Act.Sigmoid, bias=BMOD, scale=1.0)
    ps_m2 = psB.tile([1, NSTRIP, 3], f32, tag="ps_m")
    for dx in range(3):
        nc.tensor.matmul(ps_m2, lhsT=WMOD2[:, dx:dx + 1],
                         rhs=XM2[:, :, 1:2, dx:3 + dx],
                         start=(dx == 0), stop=False)
    for dx in range(3):
        nc.tensor.matmul(ps_m2, lhsT=WMOD1[:, dx:dx + 1],
                         rhs=XM2[0:64, :, 3:4, dx:3 + dx],
                         start=False, stop=(dx == 2))
    nc.scalar.activation(MODV[:, :, 96:99], ps_m2, Act.Sigmoid,
                         bias=BMOD, scale=1.0)

    # replicate mod across the 64 channel partitions (PE ones-matmul)
    MODR = work.tile([C, NSTRIP, 99], bf)
    psM1 = psB.tile([C, 297], f32, tag="ps_m")
    nc.tensor.matmul(psM1, lhsT=ONES,
                     rhs=ap(MODV, 0, [MODV[:].ap[0], [1, 297]]),
                     start=True, stop=True)
    psM2 = psB.tile([C, 297], f32, tag="ps_m")
    nc.tensor.matmul(psM2, lhsT=ONES,
                     rhs=ap(MODV, 297, [MODV[:].ap[0], [1, 297]]),
                     start=True, stop=True)
    nc.scalar.copy(ap(MODR, 0, [MODR[:].ap[0], [1, 297]]), psM1)
    nc.scalar.copy(ap(MODR, 297, [MODR[:].ap[0], [1, 297]]), psM2)

    # ---- bilinear corner weights as per-(pix,d) scalars (DVE, in the
    # gather window); bf16 outputs for the bf16 combine
    CB = work.tile([NPIX, 18], f32)
    nc.vector.tensor_scalar(CB, XY0, -1.0, None, Alu.is_ge)
    INB = work.tile([NPIX, 18], f32)
    nc.vector.scalar_tensor_tensor(INB, XY0, 96.0, CB, Alu.is_le, Alu.mult)
    W0 = work.tile([NPIX, 18], f32)
    nc.vector.tensor_scalar(W0, FXY, -1.0, 1.0, Alu.mult, Alu.add)
    A0 = work.tile([NPIX, ND], f32)
    nc.vector.tensor_mul(A0, W0[:, 0:9], INB[:, 0:9])
    A1 = work.tile([NPIX, ND], f32)
    nc.vector.tensor_mul(A1, FX, INB[:, 0:9])
    Y0 = work.tile([NPIX, ND], f32)
    nc.vector.tensor_mul(Y0, W0[:, 9:18], INB[:, 9:18])
    Y1 = work.tile([NPIX, ND], f32)
    nc.vector.tensor_mul(Y1, FY, INB[:, 9:18])
    CW = work.tile([NPIX, 4, ND], bf)
    nc.vector.tensor_mul(CW[:, 0, :], Y0, A0)   # (y0, x0)
    nc.vector.tensor_mul(CW[:, 1, :], Y1, A0)   # (y1, x0)
    nc.vector.tensor_mul(CW[:, 2, :], Y0, A1)   # (y0, x1)
    nc.vector.tensor_mul(CW[:, 3, :], Y1, A1)   # (y1, x1)

    # expand corner weights along ch on Act (idle in the gather window) so
    # the combine hits DVE 2-byte fast mode (all last dims packed)
    CWE = work.tile([NPIX, 4, ND, 64], bf)
    for c in range(4):
        nc.scalar.copy(CWE[:, c, :, :],
                       ap(CW, ND * c, [CW[:].ap[0], [1, ND], [0, 64]]))

    # ---- weighted corner combine per gather half (separate tiles so the
    # first half combines while the second gather is in flight)
    TC1 = work.tile([NPIX, 6, 64], bf)
    TC2 = work.tile([NPIX, 3, 64], bf)

    def halves(c, lo, hi, vv):
        vs = ap(vv, 64 * c, [vv[:].ap[0], [256, hi - lo], [1, 64]])
        ws = CWE[:, c, lo:hi, :]
        return vs, ws

    for (lo, hi, vv, tcx, gtag) in ((0, 6, VV1, TC1, "g1"),
                                    (6, 9, VV2, TC2, "g2")):
        n = hi - lo
        m1 = work.tile([NPIX, n, 64], bf, tag=f"m1{gtag}")
        m2 = work.tile([NPIX, n, 64], bf, tag=f"m2{gtag}")
        m3 = work.tile([NPIX, n, 64], bf, tag=f"m3{gtag}")
        v0, w0 = halves(0, lo, hi, vv)
        v1, w1 = halves(1, lo, hi, vv)
        v2, w2 = halves(2, lo, hi, vv)
        v3, w3 = halves(3, lo, hi, vv)
        nc.vector.tensor_tensor(tcx, v0, w0, Alu.mult)
        nc.vector.tensor_tensor(m1, v1, w1, Alu.mult)
        nc.vector.tensor_tensor(m2, v2, w2, Alu.mult)
        nc.vector.tensor_tensor(m3, v3, w3, Alu.mult)
        nc.vector.tensor_add(tcx, tcx, m1)
        nc.vector.tensor_add(m2, m2, m3)
        nc.vector.tensor_add(tcx, tcx, m2)

    # ---- per-d transpose to [ch, pix]
    psTA = psT.tile([C, 8, 128], bf, tag="ta")
    psTB = psT.tile([C, 1, 128], bf, tag="tb")
    for d in range(9):
        pd = psTA[:, d, :] if d < 8 else psTB[:, 0, :]
        tcs = TC1[:, d, :] if d < 6 else TC2[:, d - 6, :]
        nc.tensor.transpose(pd, tcs, IDB)

    # ---- fused scatter+modulation into FP: feat col t = 9*jj + d
    fpap = FP[:].ap[0]
    mdap = MODR[:].ap[0]
    taap = psTA[:].ap[0]
    tbap = psTB[:].ap[0]
    # d 0..5, jj 0..10 (phi=0 cols 1+9jj+d)
    nc.vector.tensor_tensor(
        ap(FP, 1, [fpap, [196, 6], [1, 6], [9, NJ]]),
        ap(psTA, 0, [taap, [NJ, 6], [128, 6], [1, NJ]]),
        ap(MODR, 0, [mdap, [99, 6], [1, 6], [9, NJ]]), Alu.mult)
    # d 6..7, jj 0..9
    nc.vector.tensor_tensor(
        ap(FP, 7, [fpap, [196, 6], [1, 2], [9, 10]]),
        ap(psTA, 6 * 128, [taap, [NJ, 6], [128, 2], [1, 10]]),
        ap(MODR, 6, [mdap, [99, 6], [1, 2], [9, 10]]), Alu.mult)
    # d 8, jj 0..9
    nc.vector.tensor_tensor(
        ap(FP, 9, [fpap, [196, 6], [9, 10]]),
        ap(psTB, 0, [tbap, [NJ, 6], [1, 10]]),
        ap(MODR, 8, [mdap, [99, 6], [9, 10]]), Alu.mult)
    # phi=1 fixups: t in {96, 97, 98} from (d, jj) = (6..8, 10)
    nc.vector.tensor_tensor(
        ap(FP, 98 + 1, [fpap, [196, 6], [1, 2]]),
        ap(psTA, 6 * 128 + 10, [taap, [NJ, 6], [128, 2]]),
        ap(MODR, 96, [mdap, [99, 6], [1, 2]]), Alu.mult)
    nc.vector.tensor_tensor(
        ap(FP, 98 + 3, [fpap, [196, 6], [1, 1]]),
        ap(psTB, 10, [tbap, [NJ, 6], [1, 1]]),
        ap(MODR, 98, [mdap, [99, 6], [1, 1]]), Alu.mult)

    # ---- final conv strips: tap-accumulate over the 2 live feat rows;
    # feat row 9s+phi feeds out row 9s+phi-dy, i.e. dst rows (1-dy):(3-dy).
    # tap order (dy=+1, dy=-1, dy=0): the first tap of each dy group
    # start=True-initializes its disjoint 2-row region (rows 0:2 then 2:4),
    # so no zero-priming matmul is needed.
    TAP_ORDER = (6, 7, 8, 0, 1, 2, 3, 4, 5)
    for s in range(NSTRIP):
        ps_c = psD.tile([C, 4, 96], f32, tag="ps_c")
        for i, t in enumerate(TAP_ORDER):
            dy, dx = t // 3 - 1, t % 3 - 1
            nc.tensor.matmul(
                ps_c[:, 1 - dy:3 - dy, :],
                lhsT=WCNV[:, t, :],
                rhs=FP[:, s, :, 1 + dx:97 + dx],
                start=(i in (0, 3)),
                stop=(i in (2, 5, 8)),
                skip_group_check=True,
            )
        if s % 2 == 0:
            OUTS2 = loop_sb.tile([C, 2, 4, 96], bf, tag="outs")
        nc.scalar.copy(OUTS2[:, s % 2, 0:2, :], ps_c[:, 0:2, :])
        nc.vector.tensor_copy(OUTS2[:, s % 2, 2:4, :], ps_c[:, 2:4, :])
        if s % 2 == 1:
            if s % 4 == 1:
                nc.sync.dma_start(out=strips_out[:, s - 1:s + 1], in_=OUTS2)
            else:
                nc.scalar.dma_start(out=strips_out[:, s - 1:s + 1], in_=OUTS2)

    # PE p-state warmers: tiny no-op matmuls the scheduler slots into PE
    # idle gaps so the tensor engine stays at full clock for the
    # transposes and final conv strips
    ps_w = psA.tile([C, 64], f32, tag="ps_off")
    with tc.high_priority(offset=-1000000):
        for _ in range(200):
            nc.tensor.matmul(ps_w, lhsT=IDB[0:64, 0:64],
                             rhs=IDB[0:64, 64:128], start=True, stop=True)

    ctx.close()


@functools.lru_cache(maxsize=1)
def _build_program():
    from contextlib import ExitStack

    import concourse.bacc as bacc
    import concourse.tile as tile
    from concourse import mybir

    dt = mybir.dt
    nc = bacc.Bacc("TRN2", target_bir_lowering=False, debug=False)
    ins = {
        "xh2": nc.dram_tensor("xh2", [XH2ROWS, 2 * C], dt.bfloat16,
                              kind="ExternalInput").ap(),
        "convw": nc.dram_tensor("convw", [128, CWCOLS], dt.float32,
                                kind="ExternalInput").ap(),
        "misc": nc.dram_tensor("misc", [128, MICOLS], dt.float32,
                               kind="ExternalInput").ap(),
        "blob16": nc.dram_tensor("blob16", [128, F16COLS], dt.bfloat16,
                                 kind="ExternalInput").ap(),
    }
    outs = {
        "strips_out": nc.dram_tensor("strips_out", [C, NSTRIP, 4, 96],
                                     dt.bfloat16, kind="ExternalOutput").ap(),
    }
    with ExitStack() as ctx:
        tc = ctx.enter_context(tile.TileContext(nc))
        emit_kernel(tc, outs, ins)
    nc.compile()
    return nc


def _host_inputs(inputs):
    arrs = {k: np.asarray(v, np.float32) for k, v in inputs.items()}
    in_maps = []
    for core in range(8):
        b, part = core // 2, core % 2
        in_maps.append(_make_core_inputs(
            arrs["x"], arrs["w_off1"], arrs["b_off1"], arrs["w_off2"],
            arrs["b_off2"], arrs["w_mod"], arrs["b_mod"],
            arrs["conv_weight"], float(arrs["alpha"][0]), b, part))
    return in_maps


def _assemble(results):
    out = np.zeros((4, C, H, W), np.float32)
    for core, res in enumerate(results):
        b, part = core // 2, core % 2
        i0 = 6 * part
        strips = np.asarray(res["strips_out"], dtype=np.float32)
        for s in range(NSTRIP):
            r0 = 9 * (i0 + s) - 1
            if r0 < 0:
                out[b][:, 0:r0 + 4, :] = strips[:, s, -r0:, :]
            elif r0 + 4 <= H:
                out[b][:, r0:r0 + 4, :] = strips[:, s]
    return out


def kernel(**inputs) -> np.ndarray:
    from concourse.bass_utils import run_bass_kernel_spmd

    nc = _build_program()
    in_maps = _host_inputs(inputs)
    res = run_bass_kernel_spmd(nc, in_maps, core_ids=list(range(8)))
    return _assemble(res.results)


if __name__ == "__main__":
    d = dict(np.load("/root/problem/inputs_cache.npz"))
    out = kernel(**d)
    ref = np.load("/root/problem/expected_np.npy")
    err = np.abs(out - ref).max()
    print("absmax err:", err, "rel:", err / np.abs(ref).max())
